# revision 4
# baseline (speedup 1.0000x reference)
"""AdaLoRA routed-LoRA kernel for 8 Trainium2 NeuronCores.

Problem (nn_AdaLoRA): per token t with expert index i:
    ds[t, :]  = slots[t, :] @ down_table[i]            # [1024] @ [1024, 16]
    out[t, :] = (ds[t, :] @ up_table[i]) / sqrt(16)    # [16] @ [16, 1024]

Sharding: data-parallel over batch (B=8 -> one batch row per core; LoRA
tables replicated on every core). Per core: 256 tokens = 2 tiles of 128
tokens (tokens on SBUF partitions). The kernel is HBM-gather bound, so
the down table is int8-quantized per (entry, rank) on the host (absmax
scaling; the f16 dequant scales, pre-multiplied by 1/sqrt(16), ride in
the last 32 bytes of each 16416-byte row so no separate gather is
needed). The up table stays f16 (it feeds TensorE, which has no int8
mode, and fp8 is too coarse for the 2e-2 gate).

Pipeline (gathers issued D0,D1,U0c*,U1c* so everything overlaps):
- down projection: one broadcast multiply (slots x int8 rows -> f16) and
  one tensor_reduce(axis=X) per 8-rank half on DVE; 4 ranks ride the
  Scalar engine's activation-accumulate path to shorten the DVE chain.
  ds is then scaled by the gathered dequant scales.
- up projection on TensorE: lhsT_h = diag(ds[:, h]) built with a single
  tensor_scalar (identity x per-partition scalar); out[t,:] accumulates
  32 matmuls [128x128]x[128x512] per tile over 4 gathered rank-chunks
  of the up rows (element_offset slices the r-major row), into one
  [128,1024] f32 PSUM tile. PSUM is copied out on the Scalar engine.
"""

import numpy as np

B, K, DIM, RANK, NE = 8, 256, 1024, 16, 4096
P = 128
N_TILE = K // P  # 2 token tiles per core
HC = 4  # up-row gather chunks per tile (4 ranks each)
RPC = RANK // HC  # ranks per up chunk
DROW = RANK * DIM  # 16384 int8 elements of down payload per row
DROWB = DROW + RANK * 2  # + 32 bytes of f16 scales
HALF = DROW // 2  # 8192: rank 0-7 payload bytes
SC_RANKS = 4  # ranks accumulated on the Scalar engine (0..3)
N_CORES = 8

_CACHE = {}


def _build():
    from concourse import bacc, bass, mybir, tile

    f32 = mybir.dt.float32
    f16 = mybir.dt.float16
    i8 = mybir.dt.int8
    i32 = mybir.dt.int32
    mult = mybir.AluOpType.mult
    add = mybir.AluOpType.add
    AX = mybir.AxisListType.X
    Copy = mybir.ActivationFunctionType.Copy

    nc = bacc.Bacc("TRN2", target_bir_lowering=False, dynamic_dma_scratch_size=65536)
    slots = nc.declare_dram_parameter("slots", [K, DIM], f16, isOutput=False)
    idx = nc.declare_dram_parameter("idx", [K, 1], i32, isOutput=False)
    downp = nc.declare_dram_parameter("downp", [NE, DROWB], i8, isOutput=False)
    up16 = nc.declare_dram_parameter("up16", [NE, DROW], f16, isOutput=False)
    ident_c = nc.declare_dram_parameter("ident_c", [P, P], f16, isOutput=False)
    out = nc.declare_dram_parameter("out", [K, DIM], f32, isOutput=True)

    with tile.TileContext(nc) as tc:
        with (
            tc.tile_pool(name="io", bufs=2) as io_pool,
            tc.tile_pool(name="dg", bufs=2) as dg_pool,
            tc.tile_pool(name="prod", bufs=1) as pr_pool,
            tc.tile_pool(name="upg", bufs=2 * HC) as up_pool,
            tc.tile_pool(name="lhs", bufs=2) as lh_pool,
            tc.tile_pool(name="misc", bufs=1) as m_pool,
            tc.tile_pool(name="ps", bufs=2, space="PSUM") as ps_pool,
        ):
            # ---- tiny index DMAs first: they gate descriptor generation ----
            idx_tiles, slots_tiles = [], []
            for t in range(N_TILE):
                tok = slice(t * P, (t + 1) * P)
                idx_t = io_pool.tile([P, 1], i32, tag="idx")
                nc.sync.dma_start(out=idx_t[:], in_=idx[tok, :])
                idx_tiles.append(idx_t)
            for t in range(N_TILE):
                tok = slice(t * P, (t + 1) * P)
                slots16 = io_pool.tile([P, DIM], f16, tag="slots16")
                nc.sync.dma_start(out=slots16[:], in_=slots[tok, :])
                slots_tiles.append(slots16)
            ident = m_pool.tile([P, P], f16)
            nc.sync.dma_start(out=ident[:], in_=ident_c[:])

            # ---- gathers: D0a D0b D1a D1b, then U0c0..3 U1c0..3 ----
            dga_tiles, dgb_tiles = [], []
            for t in range(N_TILE):
                dga = dg_pool.tile([P, HALF], i8, tag="dga")
                nc.gpsimd.indirect_dma_start(
                    out=dga[:],
                    out_offset=None,
                    in_=downp[:],
                    in_offset=bass.IndirectOffsetOnAxis(ap=idx_tiles[t][:, :1], axis=0),
                    element_offset=0,
                )
                dga_tiles.append(dga)
                dgb = dg_pool.tile([P, HALF + RANK * 2], i8, tag="dgb")
                nc.gpsimd.indirect_dma_start(
                    out=dgb[:],
                    out_offset=None,
                    in_=downp[:],
                    in_offset=bass.IndirectOffsetOnAxis(ap=idx_tiles[t][:, :1], axis=0),
                    element_offset=HALF,
                )
                dgb_tiles.append(dgb)
            upc_tiles = {}
            for t in range(N_TILE):
                for c in range(HC):
                    upc = up_pool.tile([P, RPC * DIM], f16, tag="upc")
                    nc.gpsimd.indirect_dma_start(
                        out=upc[:],
                        out_offset=None,
                        in_=up16[:],
                        in_offset=bass.IndirectOffsetOnAxis(
                            ap=idx_tiles[t][:, :1], axis=0
                        ),
                        element_offset=c * RPC * DIM,
                    )
                    upc_tiles[t, c] = upc

            # ---- down projection + diagonal lhsT build ----
            scratch = m_pool.tile([P, DIM], f16)
            lhsT_tiles, ds_parts = [], []
            for t in range(N_TILE):
                slots_b = (
                    slots_tiles[t][:]
                    .unsqueeze(1)
                    .broadcast_to([P, RANK // 2, DIM])
                )
                prod_a = pr_pool.tile([P, RANK // 2, DIM], f16, tag="prod_a")
                nc.vector.tensor_tensor(
                    out=prod_a[:],
                    in0=slots_b,
                    in1=dga_tiles[t][:].rearrange("p (r d) -> p r d", d=DIM),
                    op=mult,
                )
                prod_b = pr_pool.tile([P, RANK // 2, DIM], f16, tag="prod_b")
                nc.vector.tensor_tensor(
                    out=prod_b[:],
                    in0=slots_b,
                    in1=dgb_tiles[t][:, :HALF].rearrange("p (r d) -> p r d", d=DIM),
                    op=mult,
                )
                # ranks 0..3 accumulate on the Scalar engine
                dsacc = io_pool.tile([P, SC_RANKS], f32, tag="dsacc")
                for j in range(SC_RANKS):
                    nc.scalar.activation(
                        out=scratch[:],
                        in_=prod_a[:, j, :],
                        func=Copy,
                        accum_out=dsacc[:, j : j + 1],
                    )
                # ranks 4..15 reduce on DVE at 2x (f16 in/out)
                ds16 = io_pool.tile([P, RANK], f16, tag="ds16")
                with nc.allow_low_precision(
                    reason="ds fits f16; reduction accumulates in fp32 internally"
                ):
                    nc.vector.tensor_reduce(
                        out=ds16[:, SC_RANKS : RANK // 2],
                        in_=prod_a[:, SC_RANKS:, :],
                        axis=AX,
                        op=add,
                    )
                    nc.vector.tensor_reduce(
                        out=ds16[:, RANK // 2 :],
                        in_=prod_b[:],
                        axis=AX,
                        op=add,
                    )
                # scale by the gathered dequant scales (include 1/sqrt(16))
                scales = dgb_tiles[t][:, HALF:].bitcast(f16)
                dss = io_pool.tile([P, RANK], f32, tag="dss")
                nc.vector.tensor_tensor(
                    out=dss[:, :SC_RANKS],
                    in0=dsacc[:],
                    in1=scales[:, :SC_RANKS],
                    op=mult,
                )
                nc.vector.tensor_tensor(
                    out=dss[:, SC_RANKS:],
                    in0=ds16[:, SC_RANKS:],
                    in1=scales[:, SC_RANKS:],
                    op=mult,
                )
                # lhsT_h = diag(dss[:, h]) via identity x per-partition scalar
                lhsT = lh_pool.tile([P, RANK, P], f16, tag="lhsT")
                for h in range(RANK):
                    nc.vector.tensor_scalar(
                        out=lhsT[:, h, :],
                        in0=ident[:],
                        scalar1=dss[:, h : h + 1],
                        scalar2=None,
                        op0=mult,
                    )
                lhsT_tiles.append(lhsT)

            # ---- up projection on TensorE + output ----
            for t in range(N_TILE):
                tok = slice(t * P, (t + 1) * P)
                out_psum = ps_pool.tile([P, DIM], f32, space="PSUM", tag="outp")
                for c in range(HC):
                    upc = upc_tiles[t, c]
                    for h in range(RPC):
                        for n in range(2):
                            n0 = n * 512
                            nc.tensor.matmul(
                                out=out_psum[:, n0 : n0 + 512],
                                lhsT=lhsT_tiles[t][:, c * RPC + h, :],
                                rhs=upc[:, h * DIM + n0 : h * DIM + n0 + 512],
                                start=(c == 0 and h == 0),
                                stop=(c == HC - 1 and h == RPC - 1),
                            )
                out_sb = io_pool.tile([P, DIM], f32, tag="osb")
                nc.scalar.copy(out_sb[:], out_psum[:])
                nc.sync.dma_start(out=out[tok, :], in_=out_sb[:])
    nc.compile()
    return nc


def _get_nc():
    if "nc" not in _CACHE:
        _CACHE["nc"] = _build()
    return _CACHE["nc"]


def _prep_in_maps(slots, indices, down_proj_values, up_proj_values):
    slots = np.ascontiguousarray(np.asarray(slots, dtype=np.float32).astype(np.float16))
    indices = np.ascontiguousarray(np.asarray(indices).astype(np.int32))
    dT = np.asarray(down_proj_values, dtype=np.float32).transpose(0, 2, 1)  # [NE,R,D]
    absmax = np.abs(dT).max(axis=2, keepdims=True)
    sq = np.maximum(absmax, 1e-20) / 127.0
    q = np.clip(np.rint(dT / sq), -127, 127).astype(np.int8).reshape(NE, DROW)
    scales = (sq[:, :, 0] * 0.25).astype(np.float16)  # fold 1/sqrt(RANK)
    downp = np.ascontiguousarray(
        np.concatenate([q, scales.view(np.int8)], axis=1)
    )  # [NE, 16416] int8
    up16 = np.ascontiguousarray(
        np.asarray(up_proj_values, dtype=np.float32).reshape(NE, DROW).astype(np.float16)
    )
    ident_c = np.eye(P, dtype=np.float16)
    assert slots.shape == (B, K, DIM) and indices.shape == (B, K)
    in_maps = []
    for i in range(N_CORES):
        in_maps.append(
            {
                "slots": slots[i],
                "idx": indices[i].reshape(K, 1),
                "downp": downp,
                "up16": up16,
                "ident_c": ident_c,
            }
        )
    return in_maps


def _run(in_maps, trace=False):
    from concourse.bass_utils import run_bass_kernel_spmd

    nc = _get_nc()
    return run_bass_kernel_spmd(
        nc, in_maps, core_ids=list(range(N_CORES)), trace=trace
    )


def kernel(slots, indices, down_proj_values, up_proj_values):
    in_maps = _prep_in_maps(slots, indices, down_proj_values, up_proj_values)
    res = _run(in_maps)
    out = np.stack([res.results[i]["out"] for i in range(N_CORES)], axis=0)
    return out.astype(np.float32)


# revision 8
# speedup vs baseline: 1.5554x; 1.5554x over previous
"""AdaLoRA routed-LoRA kernel for 8 Trainium2 NeuronCores.

Problem (nn_AdaLoRA): per token t with expert index i:
    ds[t, :]  = slots[t, :] @ down_table[i]            # [1024] @ [1024, 16]
    out[t, :] = (ds[t, :] @ up_table[i]) / sqrt(16)    # [16] @ [16, 1024]

Sharding: data-parallel over batch (B=8 -> one batch row per core; LoRA
tables replicated on every core, f16). Per core: 256 tokens = 2 tiles
of 128 tokens (tokens on SBUF partitions). The kernel is HBM-gather
bound (~16MB of f16 table rows per core), so the structure keeps the 16
DMA queues saturated end to end and hides all compute under the gather:

- gather issue order D0a D0b D1a D1b U0c0..3 U1c0..3 (indirect DMAs with
  element_offset slicing the r-major rows into rank chunks), so tile 0's
  matmuls start ~halfway through the gather and tile 1's data arrives
  just in time.
- down projection per rank on measured-fast primitives: 8 ranks fused
  multiply+accumulate on DVE (scalar_tensor_tensor), 8 ranks as DVE
  multiply + Scalar-engine activation-accumulate, both accumulating f32.
  1/sqrt(16) is folded into slots on the host.
- up projection on TensorE: lhsT_h = diag(ds[:, h]) built with a single
  tensor_scalar (identity x per-partition f32 scalar); out[t,:]
  accumulates 32 matmuls [128x128]x[128x512] per tile over the 4
  gathered rank-chunks into one [128,1024] f32 PSUM tile, copied out on
  the Scalar engine.
"""

import numpy as np

B, K, DIM, RANK, NE = 8, 256, 1024, 16, 4096
P = 128
N_TILE = K // P  # 2 token tiles per core
HC = 4  # up-row gather chunks per tile (4 ranks each)
RPC = RANK // HC  # ranks per up chunk
DROW = RANK * DIM  # 16384 f16 elements per table row
HALF = DROW // 2  # 8192 elements: ranks 0-7
SCALE = 1.0 / 4.0  # 1/sqrt(RANK), folded into slots host-side
N_CORES = 8

_CACHE = {}


def _build():
    from concourse import bacc, bass, mybir, tile

    f32 = mybir.dt.float32
    f16 = mybir.dt.float16
    i32 = mybir.dt.int32
    mult = mybir.AluOpType.mult
    Copy = mybir.ActivationFunctionType.Copy

    nc = bacc.Bacc("TRN2", target_bir_lowering=False, dynamic_dma_scratch_size=65536)
    slots = nc.declare_dram_parameter("slots", [K, DIM], f16, isOutput=False)
    idx = nc.declare_dram_parameter("idx", [K, 1], i32, isOutput=False)
    down16 = nc.declare_dram_parameter("down16", [NE, DROW], f16, isOutput=False)
    up16 = nc.declare_dram_parameter("up16", [NE, DROW], f16, isOutput=False)
    ident_c = nc.declare_dram_parameter("ident_c", [P, P], f16, isOutput=False)
    out = nc.declare_dram_parameter("out", [K, DIM], f32, isOutput=True)

    with tile.TileContext(nc) as tc:
        with (
            tc.tile_pool(name="io", bufs=2) as io_pool,
            tc.tile_pool(name="dg", bufs=2) as dg_pool,
            tc.tile_pool(name="prod", bufs=2) as pr_pool,
            tc.tile_pool(name="upg", bufs=6) as up_pool,
            tc.tile_pool(name="lhs", bufs=2) as lh_pool,
            tc.tile_pool(name="misc", bufs=1) as m_pool,
            tc.tile_pool(name="ob", bufs=1) as ob_pool,
            tc.tile_pool(name="ps", bufs=2, space="PSUM") as ps_pool,
        ):
            # ---- tiny index DMAs first: they gate descriptor generation ----
            idx_tiles, slots_tiles = [], []
            for t in range(N_TILE):
                tok = slice(t * P, (t + 1) * P)
                idx_t = io_pool.tile([P, 1], i32, tag="idx")
                nc.sync.dma_start(out=idx_t[:], in_=idx[tok, :])
                idx_tiles.append(idx_t)
            for t in range(N_TILE):
                tok = slice(t * P, (t + 1) * P)
                slots16 = io_pool.tile([P, DIM], f16, tag="slots16")
                nc.sync.dma_start(out=slots16[:], in_=slots[tok, :])
                slots_tiles.append(slots16)
            ident = m_pool.tile([P, P], f16)
            nc.sync.dma_start(out=ident[:], in_=ident_c[:])

            # ---- gathers: D0a D0b D1a D1b, then U0c0..3 U1c0..3 ----
            dg_tiles = {}
            for t in range(N_TILE):
                for h2 in range(2):
                    dg = dg_pool.tile([P, HALF], f16, tag=f"dg{h2}")
                    nc.gpsimd.indirect_dma_start(
                        out=dg[:],
                        out_offset=None,
                        in_=down16[:],
                        in_offset=bass.IndirectOffsetOnAxis(
                            ap=idx_tiles[t][:, :1], axis=0
                        ),
                        element_offset=h2 * HALF,
                    )
                    dg_tiles[t, h2] = dg
            upc_tiles = {}
            for t in range(N_TILE):
                for c in range(HC):
                    upc = up_pool.tile([P, RPC * DIM], f16, tag="upc")
                    nc.gpsimd.indirect_dma_start(
                        out=upc[:],
                        out_offset=None,
                        in_=up16[:],
                        in_offset=bass.IndirectOffsetOnAxis(
                            ap=idx_tiles[t][:, :1], axis=0
                        ),
                        element_offset=c * RPC * DIM,
                    )
                    upc_tiles[t, c] = upc

            # ---- down projection + diagonal lhsT build ----
            scratch = m_pool.tile([P, DIM], f16)
            scratch2 = m_pool.tile([P, DIM], f16)
            lhsT_tiles = []
            for t in range(N_TILE):
                slots16 = slots_tiles[t]
                ds32 = io_pool.tile([P, RANK], f32, tag="ds32")
                for h2 in range(2):
                    dch = dg_tiles[t, h2][:].rearrange("p (r d) -> p r d", d=DIM)
                    # even local ranks: DVE multiply feeding Scalar accumulate
                    prods = []
                    for rl in range(0, RANK // 2, 2):
                        prod = pr_pool.tile([P, DIM], f16, tag=f"prod{rl // 2}")
                        nc.vector.tensor_tensor(
                            out=prod[:], in0=slots16[:], in1=dch[:, rl, :], op=mult
                        )
                        prods.append((h2 * (RANK // 2) + rl, prod))
                    # odd local ranks: fused multiply+accumulate on DVE
                    for rl in range(1, RANK // 2, 2):
                        r = h2 * (RANK // 2) + rl
                        nc.vector.scalar_tensor_tensor(
                            out=scratch[:],
                            in0=slots16[:],
                            scalar=1.0,
                            in1=dch[:, rl, :],
                            op0=mult,
                            op1=mult,
                            accum_out=ds32[:, r : r + 1],
                        )
                    for r, prod in prods:
                        nc.scalar.activation(
                            out=scratch2[:],
                            in_=prod[:],
                            func=Copy,
                            accum_out=ds32[:, r : r + 1],
                        )
                # lhsT_h = diag(ds32[:, h]) via identity x per-partition scalar
                lhsT = lh_pool.tile([P, RANK, P], f16, tag="lhsT")
                for h in range(RANK):
                    nc.vector.tensor_scalar(
                        out=lhsT[:, h, :],
                        in0=ident[:],
                        scalar1=ds32[:, h : h + 1],
                        scalar2=None,
                        op0=mult,
                    )
                lhsT_tiles.append(lhsT)

            # ---- up projection on TensorE + output ----
            for t in range(N_TILE):
                tok = slice(t * P, (t + 1) * P)
                out_psum = ps_pool.tile([P, DIM], f32, space="PSUM", tag="outp")
                for c in range(HC):
                    upc = upc_tiles[t, c]
                    for h in range(RPC):
                        for n in range(2):
                            n0 = n * 512
                            nc.tensor.matmul(
                                out=out_psum[:, n0 : n0 + 512],
                                lhsT=lhsT_tiles[t][:, c * RPC + h, :],
                                rhs=upc[:, h * DIM + n0 : h * DIM + n0 + 512],
                                start=(c == 0 and h == 0),
                                stop=(c == HC - 1 and h == RPC - 1),
                            )
                out_sb = ob_pool.tile([P, DIM], f32, tag="osb")
                nc.scalar.copy(out_sb[:], out_psum[:])
                nc.sync.dma_start(out=out[tok, :], in_=out_sb[:])
    nc.compile()
    return nc


def _get_nc():
    if "nc" not in _CACHE:
        _CACHE["nc"] = _build()
    return _CACHE["nc"]


def _prep_in_maps(slots, indices, down_proj_values, up_proj_values):
    slots = np.ascontiguousarray(
        (np.asarray(slots, dtype=np.float32) * SCALE).astype(np.float16)
    )
    indices = np.ascontiguousarray(np.asarray(indices).astype(np.int32))
    down16 = np.ascontiguousarray(
        np.asarray(down_proj_values, dtype=np.float32)
        .transpose(0, 2, 1)
        .reshape(NE, DROW)
        .astype(np.float16)
    )
    up16 = np.ascontiguousarray(
        np.asarray(up_proj_values, dtype=np.float32).reshape(NE, DROW).astype(np.float16)
    )
    ident_c = np.eye(P, dtype=np.float16)
    assert slots.shape == (B, K, DIM) and indices.shape == (B, K)
    in_maps = []
    for i in range(N_CORES):
        in_maps.append(
            {
                "slots": slots[i],
                "idx": indices[i].reshape(K, 1),
                "down16": down16,
                "up16": up16,
                "ident_c": ident_c,
            }
        )
    return in_maps


def _run(in_maps, trace=False):
    from concourse.bass_utils import run_bass_kernel_spmd

    nc = _get_nc()
    return run_bass_kernel_spmd(
        nc, in_maps, core_ids=list(range(N_CORES)), trace=trace
    )


def kernel(slots, indices, down_proj_values, up_proj_values):
    in_maps = _prep_in_maps(slots, indices, down_proj_values, up_proj_values)
    res = _run(in_maps)
    out = np.stack([res.results[i]["out"] for i in range(N_CORES)], axis=0)
    return out.astype(np.float32)
